# revision 25
# baseline (speedup 1.0000x reference)
"""AnchorTargetLayer max-IoU kernel for 8 TRN2 NeuronCores.

max_iou[b, n] = max_g IoU(anchor_n, gt_box[b, g]);
anchors [100000, 4] f32, gt_boxes [4, 64, 4] f32 -> out [4, 100000] f32.

Sharding: anchors split 8 ways (12544/core incl pad), gt replicated,
no collectives. Per-core layout: anchors on SBUF partitions (128 per
block, 98 blocks), all B*G = 256 (batch, gt) pairs on the free dim.
GT-derived rows are broadcast once into [128, 5*256] SBUF (loop
invariant); per-anchor coords are [128,1] per-partition scalars.

Math per (anchor, pair):
  iw   = min(ax2, gx2) + min(-ax1, -gx1)        (x-overlap, may be <0)
  ih   = likewise in y
  inter = max(iw, eps) * max(ih, eps)
  v    = (areaA + areaG) * (1/inter)            (= union/inter + 1)
  vmin[b] = min_g v ;  out = 1/(min(vmin,1e30) - 1)

The reciprocal runs on the Scalar engine (Activation Reciprocal,
emitted directly); everything else on Vector. DVE and ACT are
software-pipelined one block apart with two semaphores.
"""

import os
import sys

import numpy as np

sys.path.insert(0, "/opt/trn_rl_repo")

import concourse.bass as bass
import concourse.mybir as mybir
from concourse.bass_utils import run_bass_kernel_spmd

N_ANCHORS = 100000
BATCH = 4
N_GT = 64
N_CORES = 8

P = 128
BLOCKS = 98
N_LOC = P * BLOCKS          # 12544
N_PAD = N_LOC * N_CORES     # 100352
NPAIR = BATCH * N_GT        # 256

F32 = mybir.dt.float32
F16 = mybir.dt.float16
EPS = 1e-15

LAST_EXEC_NS = None


def _ensure_axon_ntff_hook():
    try:
        import antenv.axon_hooks  # noqa: F401

        return
    except ImportError:
        pass
    import contextlib
    import ctypes
    import types

    import antenv

    m = types.ModuleType("antenv.axon_hooks")
    m._hook = None

    def set_axon_ntff_profile_hook(h):
        m._hook = h

    def get_axon_ntff_profile_hook():
        return m._hook

    m.set_axon_ntff_profile_hook = set_axon_ntff_profile_hook
    m.get_axon_ntff_profile_hook = get_axon_ntff_profile_hook
    sys.modules["antenv.axon_hooks"] = m
    antenv.axon_hooks = m

    so_path = os.environ.get("PJRT_LIBRARY_PATH", "/opt/axon/libaxon_pjrt.so")
    try:
        lib = ctypes.CDLL(so_path)
    except OSError:
        return
    if not hasattr(lib, "axon_start_nrt_profile"):
        return
    lib.axon_start_nrt_profile.argtypes = [
        ctypes.POINTER(ctypes.c_int64),
        ctypes.c_size_t,
    ]
    lib.axon_start_nrt_profile.restype = ctypes.c_int64
    lib.axon_stop_nrt_profile.argtypes = [ctypes.c_char_p]
    lib.axon_stop_nrt_profile.restype = ctypes.c_int64

    @contextlib.contextmanager
    def _hook(output_dir, device_ids):
        import jax

        jax.devices()
        if device_ids:
            ids = (ctypes.c_int64 * len(device_ids))(*device_ids)
            rc = lib.axon_start_nrt_profile(ids, len(device_ids))
        else:
            rc = lib.axon_start_nrt_profile(None, 0)
        if rc != 0:
            raise RuntimeError(f"axon_start_nrt_profile rc={rc}")
        try:
            yield
        finally:
            n = lib.axon_stop_nrt_profile(str(output_dir).encode())
            if n < 0:
                raise RuntimeError(f"axon_stop_nrt_profile rc={n}")

    set_axon_ntff_profile_hook(_hook)


def _patch_upload_artifacts():
    import concourse.bass_utils as bu

    if getattr(bu.upload_artifacts, "_safe", False):
        return
    orig = bu.upload_artifacts

    def safe(tmpdir):
        try:
            return orig(tmpdir)
        except Exception:
            return tmpdir

    safe._safe = True
    bu.upload_artifacts = safe


def _act_recip(scalar_eng, nc, out_ap, in_ap, bias=0.0):
    """Directly emit Activation(Reciprocal) (the nc.scalar.activation wrapper
    rejects Reciprocal)."""
    ins = [scalar_eng.lower_ap(in_ap)]
    for argv in (bias, 1.0, 0.0):  # bias, scale, alpha
        ins.append(mybir.ImmediateValue(dtype=F32, value=argv))
    return scalar_eng.add_instruction(
        mybir.InstActivation(
            name=nc.get_next_instruction_name(),
            func=mybir.ActivationFunctionType.Reciprocal,
            ins=ins,
            outs=[scalar_eng.lower_ap(out_ap)],
        )
    )


class _Ticks:
    """Per-engine completion tick bookkeeping for cross-engine waits.

    Tick numbers are precomputed from the schedule (so any engine stream can
    be emitted first); each tracked instruction gets `.then_inc(sem, 1)`;
    waiters use wait_ge(sem, tick).
    """

    def __init__(self, orders, sems):
        # orders: {eng_name: [key, ...]} in emission order; sems: {eng: sem}
        self.tick_no = {}
        self.key_eng = {}
        for eng, keys in orders.items():
            for t, key in enumerate(keys, start=1):
                self.tick_no[key] = t
                self.key_eng[key] = eng
        self.sems = sems

    def mark(self, inst, key):
        inst.then_inc(self.sems[self.key_eng[key]], 1)

    def wait(self, engine, key):
        engine.wait_ge(self.sems[self.key_eng[key]], self.tick_no[key])


def _build_graph():
    nc = bass.Bass()
    A_ext = nc.declare_dram_parameter("anchors_p", [P, BLOCKS * 4], F32, isOutput=False)
    AR_ext = nc.declare_dram_parameter("aarea", [P, BLOCKS], F32, isOutput=False)
    GT_ext = nc.declare_dram_parameter("gtrows", [5, NPAIR], F32, isOutput=False)
    GTH_ext = nc.declare_dram_parameter("gtrows_h", [2, NPAIR], F16, isOutput=False)
    out_ext = nc.declare_dram_parameter("out", [P, BLOCKS * 4], F32, isOutput=True)

    Alu = mybir.AluOpType
    PAIRS = BLOCKS // 2  # 49 block pairs; per-block buffers have 4 slots

    with (
        nc.sbuf_tensor("A", [P, BLOCKS * 4], F32) as A,
        nc.sbuf_tensor("AR", [P, BLOCKS], F32) as AR,
        nc.sbuf_tensor("GTB", [P, 5, NPAIR], F32) as GTB,
        nc.sbuf_tensor("GTH", [P, 2, NPAIR], F16) as GTH,
        nc.sbuf_tensor("TX", [P, NPAIR], F32) as TX,
        nc.sbuf_tensor("TY", [P, NPAIR], F32) as TY,
        nc.sbuf_tensor("SX4", [P, 3, 2, NPAIR], F32) as SX4,
        nc.sbuf_tensor("SY4", [P, 3, 2, NPAIR], F32) as SY4,
        nc.sbuf_tensor("IHR4", [P, 3, 2, NPAIR], F32) as IHR4,
        nc.sbuf_tensor("INT4", [P, 2, 2, NPAIR], F32) as INT4,
        nc.sbuf_tensor("RI4", [P, 2, 2, NPAIR], F32) as RI4,
        nc.sbuf_tensor("SA8", [P, 4, 2, NPAIR], F32) as SA8,
        nc.sbuf_tensor("VB4", [P, 2, 2, NPAIR], F32) as VB4,
        nc.sbuf_tensor("VOUT", [P, BLOCKS * 4], F32) as VOUT,
        nc.sbuf_tensor("MIOU", [P, BLOCKS * 4], F32) as MIOU,
        nc.Block() as block,
        nc.semaphore("dma_sem") as dma_sem,
        nc.semaphore("dve_sem") as dve_sem,
        nc.semaphore("act_sem") as act_sem,
        nc.semaphore("pool_sem") as pool_sem,
    ):
        # ---- schedule (mirrors emission exactly) ----
        dve_order = []
        for k in range(PAIRS + 5):
            if k < PAIRS:
                dve_order += [("sy", 2 * k), ("sy", 2 * k + 1)]
            if 2 <= k < PAIRS + 2:
                p = k - 2
                dve_order += [("inter", 2 * p), ("inter", 2 * p + 1)]
            if 5 <= k < PAIRS + 5:
                dve_order.append(("red2", k - 5))
        dve_order.append(("vc", 0))
        act_order = []
        for k in range(PAIRS + 5):
            if 1 <= k < PAIRS + 1:
                p = k - 1
                act_order += [("ihr2", p), ("sa", 2 * p), ("sa", 2 * p + 1)]
            if 3 <= k < PAIRS + 3:
                act_order.append(("rint2", k - 3))
        act_order.append(("miou", 0))
        pool_order = []
        for k in range(PAIRS + 5):
            if 4 <= k < PAIRS + 4:
                p = k - 4
                pool_order += [("v", 2 * p), ("v", 2 * p + 1)]

        tk = _Ticks(
            {"dve": dve_order, "act": act_order, "pool": pool_order},
            {"dve": dve_sem, "act": act_sem, "pool": pool_sem},
        )

        @block.sync
        def _(sync):
            sync.dma_start(out=A[:, :], in_=A_ext[:, :]).then_inc(dma_sem, 16)
            sync.dma_start(out=AR[:, :], in_=AR_ext[:, :]).then_inc(dma_sem, 16)
            g_ap = GT_ext[:, :]
            g_b = bass.AP(
                tensor=g_ap.tensor, offset=g_ap.offset, ap=[[0, P]] + list(g_ap.ap)
            )
            sync.dma_start(out=GTB[:, :, :], in_=g_b).then_inc(dma_sem, 16)
            h_ap = GTH_ext[:, :]
            h_b = bass.AP(
                tensor=h_ap.tensor, offset=h_ap.offset, ap=[[0, P]] + list(h_ap.ap)
            )
            sync.dma_start(out=GTH[:, :, :], in_=h_b).then_inc(dma_sem, 16)

        GX1N = GTB[:, 0, :]
        GX2 = GTB[:, 1, :]
        GY1N = GTB[:, 2, :]
        GY2 = GTB[:, 3, :]
        GAREA = GTB[:, 4, :]

        def slot(j):
            return ((j % 4) // 2, (j % 4) % 2)

        def dve_front(vector, j):
            nax1 = A[:, 4 * j + 0 : 4 * j + 1]
            ax2 = A[:, 4 * j + 1 : 4 * j + 2]
            nay1 = A[:, 4 * j + 2 : 4 * j + 3]
            ay2 = A[:, 4 * j + 3 : 4 * j + 4]
            h, l = (j // 2) % 3, j % 2
            vector.tensor_scalar(
                out=TX[:, :], in0=GX2, scalar1=ax2, scalar2=None, op0=Alu.min
            )
            vector.scalar_tensor_tensor(
                out=SX4[:, h, l, :], in0=GX1N, scalar=nax1, in1=TX[:, :],
                op0=Alu.min, op1=Alu.add,
            )
            vector.tensor_scalar(
                out=TY[:, :], in0=GY2, scalar1=ay2, scalar2=None, op0=Alu.min
            )
            i = vector.scalar_tensor_tensor(
                out=SY4[:, h, l, :], in0=GY1N, scalar=nay1, in1=TY[:, :],
                op0=Alu.min, op1=Alu.add,
            )
            tk.mark(i, ("sy", j))

        def dve_inter(vector, j):
            h3, l = (j // 2) % 3, j % 2
            h2 = (j // 2) % 2
            i = vector.scalar_tensor_tensor(
                out=INT4[:, h2, l, :], in0=SX4[:, h3, l, :], scalar=0.0,
                in1=IHR4[:, h3, l, :], op0=Alu.max, op1=Alu.mult,
            )
            tk.mark(i, ("inter", j))

        def dve_red2(vector, k):
            h = k % 2
            tk.wait(vector, ("v", 2 * k + 1))
            i = vector.tensor_reduce(
                out=VOUT[:, 8 * k : 8 * (k + 1)],
                in_=VB4[:, h, :, :].rearrange("p bl (bt g) -> p bl bt g", bt=BATCH),
                axis=mybir.AxisListType.X,
                op=Alu.min,
            )
            tk.mark(i, ("red2", k))

        def act_mid(scalar, p):
            h = p % 3
            tk.wait(scalar, ("sy", 2 * p + 1))
            if p >= 3:
                tk.wait(scalar, ("inter", 2 * p - 5))  # IHR slab free (3 deep)
            i = scalar.activation(
                out=IHR4[:, h, :, :], in_=SY4[:, h, :, :],
                func=mybir.ActivationFunctionType.Relu,
            )
            tk.mark(i, ("ihr2", p))
            for j in (2 * p, 2 * p + 1):
                if j >= 8:
                    tk.wait(scalar, ("v", j - 8))  # SA slot free (8 deep)
                hj, lj = (j % 8) // 2, j % 2
                i = scalar.activation(
                    out=SA8[:, hj, lj, :], in_=GAREA,
                    func=mybir.ActivationFunctionType.Identity,
                    bias=AR[:, j : j + 1], scale=1.0,
                )
                tk.mark(i, ("sa", j))

        def act_rint2(scalar, k):
            h = k % 2
            tk.wait(scalar, ("inter", 2 * k + 1))
            if k >= 2:
                tk.wait(scalar, ("v", 2 * (k - 2) + 1))  # RI slab free
            i = _act_recip(scalar, nc, RI4[:, h, :, :], INT4[:, h, :, :], bias=1e-12)
            tk.mark(i, ("rint2", k))

        def pool_v(gpsimd, j):
            p = j // 2
            h, l = slot(j)
            if j % 2 == 0:
                tk.wait(gpsimd, ("rint2", p))  # also covers sa(j)
                if p >= 2:
                    tk.wait(gpsimd, ("red2", p - 2))  # VB slab free
            i = gpsimd.tensor_tensor(
                out=VB4[:, h, l, :], in0=SA8[:, (j % 8) // 2, j % 2, :],
                in1=RI4[:, h, l, :], op=Alu.mult,
            )
            tk.mark(i, ("v", j))

        @block.vector
        def _(vector):
            vector.wait_ge(dma_sem, 64)
            for k in range(PAIRS + 5):
                if k < PAIRS:
                    if k >= 3:
                        tk.wait(vector, ("ihr2", k - 3))  # SY slab free (3 deep)
                    dve_front(vector, 2 * k)
                    dve_front(vector, 2 * k + 1)
                if 2 <= k < PAIRS + 2:
                    p = k - 2
                    tk.wait(vector, ("ihr2", p))
                    if p >= 2:
                        tk.wait(vector, ("rint2", p - 2))  # INT slab free
                    dve_inter(vector, 2 * p)
                    dve_inter(vector, 2 * p + 1)
                if 5 <= k < PAIRS + 5:
                    dve_red2(vector, k - 5)
            i = vector.tensor_scalar(
                out=MIOU[:, :], in0=VOUT[:, :], scalar1=1e30, scalar2=-1.0,
                op0=Alu.min, op1=Alu.add,
            )
            tk.mark(i, ("vc", 0))

        @block.scalar
        def _(scalar):
            for k in range(PAIRS + 5):
                if 1 <= k < PAIRS + 1:
                    act_mid(scalar, k - 1)
                if 3 <= k < PAIRS + 3:
                    act_rint2(scalar, k - 3)
            tk.wait(scalar, ("vc", 0))
            i = _act_recip(scalar, nc, MIOU[:, :], MIOU[:, :])
            tk.mark(i, ("miou", 0))

        @block.gpsimd
        def _(gpsimd):
            for k in range(PAIRS + 5):
                if 4 <= k < PAIRS + 4:
                    p = k - 4
                    pool_v(gpsimd, 2 * p)
                    pool_v(gpsimd, 2 * p + 1)

        @block.sync
        def _(sync):
            tk.wait(sync, ("miou", 0))
            sync.dma_start(out=out_ext[:, :], in_=MIOU[:, :]).then_inc(dma_sem, 16)
            sync.wait_ge(dma_sem, 80)

    return nc


def kernel(anchors: np.ndarray, gt_boxes: np.ndarray) -> np.ndarray:
    global LAST_EXEC_NS
    anchors = np.asarray(anchors, dtype=np.float32)
    gt_boxes = np.asarray(gt_boxes, dtype=np.float32)

    apad = np.zeros((N_PAD, 4), dtype=np.float32)
    apad[:N_ANCHORS] = anchors

    g = gt_boxes.reshape(NPAIR, 4)
    garea = (g[:, 2] - g[:, 0]) * (g[:, 3] - g[:, 1])
    gtrows = np.stack([-g[:, 0], g[:, 2], -g[:, 1], g[:, 3], garea]).astype(np.float32)
    gtrows = np.ascontiguousarray(gtrows)
    gtrows_h = np.ascontiguousarray(np.stack([g[:, 2], g[:, 3]]).astype(np.float16))

    in_maps = []
    for c in range(N_CORES):
        sh = apad[c * N_LOC : (c + 1) * N_LOC]
        a3 = sh.reshape(P, BLOCKS, 4)
        ap = np.empty_like(a3)
        ap[:, :, 0] = -a3[:, :, 0]
        ap[:, :, 1] = a3[:, :, 2]
        ap[:, :, 2] = -a3[:, :, 1]
        ap[:, :, 3] = a3[:, :, 3]
        aarea = (a3[:, :, 2] - a3[:, :, 0]) * (a3[:, :, 3] - a3[:, :, 1])
        in_maps.append(
            {
                "anchors_p": np.ascontiguousarray(ap.reshape(P, BLOCKS * 4)),
                "aarea": np.ascontiguousarray(aarea.astype(np.float32)),
                "gtrows": gtrows,
                "gtrows_h": gtrows_h,
            }
        )

    nc = _build_graph()
    trace = os.environ.get("ANCHOR_TRACE", "0") == "1"
    core_ids = list(range(N_CORES))
    if trace:
        _ensure_axon_ntff_hook()
        _patch_upload_artifacts()
        try:
            res = run_bass_kernel_spmd(nc, in_maps, core_ids=core_ids, trace=True)
        except Exception as e:
            print(f"trace run failed ({type(e).__name__}: {e}); falling back", file=sys.stderr)
            res = run_bass_kernel_spmd(nc, in_maps, core_ids=core_ids, trace=False)
    else:
        res = run_bass_kernel_spmd(nc, in_maps, core_ids=core_ids, trace=False)
    LAST_EXEC_NS = res.exec_time_ns

    out = np.empty((BATCH, N_PAD), dtype=np.float32)
    for c in range(N_CORES):
        o = res.results[c]["out"].reshape(P, BLOCKS, 4)
        out[:, c * N_LOC : (c + 1) * N_LOC] = o.transpose(2, 0, 1).reshape(BATCH, N_LOC)
    return out[:, :N_ANCHORS]


# revision 26
# speedup vs baseline: 1.1190x; 1.1190x over previous
"""AnchorTargetLayer max-IoU kernel for 8 TRN2 NeuronCores.

max_iou[b, n] = max_g IoU(anchor_n, gt_box[b, g]);
anchors [100000, 4] f32, gt_boxes [4, 64, 4] f32 -> out [4, 100000] f32.

Sharding: anchors split 8 ways (12544/core incl pad), gt replicated,
no collectives. Per-core layout: anchors on SBUF partitions (128 per
block, 98 blocks), all B*G = 256 (batch, gt) pairs on the free dim.
GT-derived rows are broadcast once into [128, 5*256] SBUF (loop
invariant); per-anchor coords are [128,1] per-partition scalars.

Math per (anchor, pair):
  iw   = min(ax2, gx2) + min(-ax1, -gx1)        (x-overlap, may be <0)
  ih   = likewise in y
  inter = max(iw, eps) * max(ih, eps)
  v    = (areaA + areaG) * (1/inter)            (= union/inter + 1)
  vmin[b] = min_g v ;  out = 1/(min(vmin,1e30) - 1)

The reciprocal runs on the Scalar engine (Activation Reciprocal,
emitted directly); everything else on Vector. DVE and ACT are
software-pipelined one block apart with two semaphores.
"""

import os
import sys

import numpy as np

sys.path.insert(0, "/opt/trn_rl_repo")

import concourse.bass as bass
import concourse.mybir as mybir
from concourse.bass_utils import run_bass_kernel_spmd

N_ANCHORS = 100000
BATCH = 4
N_GT = 64
N_CORES = 8

P = 128
BLOCKS = 98
N_LOC = P * BLOCKS          # 12544
N_PAD = N_LOC * N_CORES     # 100352
NPAIR = BATCH * N_GT        # 256

F32 = mybir.dt.float32
EPS = 1e-15

LAST_EXEC_NS = None


def _ensure_axon_ntff_hook():
    try:
        import antenv.axon_hooks  # noqa: F401

        return
    except ImportError:
        pass
    import contextlib
    import ctypes
    import types

    import antenv

    m = types.ModuleType("antenv.axon_hooks")
    m._hook = None

    def set_axon_ntff_profile_hook(h):
        m._hook = h

    def get_axon_ntff_profile_hook():
        return m._hook

    m.set_axon_ntff_profile_hook = set_axon_ntff_profile_hook
    m.get_axon_ntff_profile_hook = get_axon_ntff_profile_hook
    sys.modules["antenv.axon_hooks"] = m
    antenv.axon_hooks = m

    so_path = os.environ.get("PJRT_LIBRARY_PATH", "/opt/axon/libaxon_pjrt.so")
    try:
        lib = ctypes.CDLL(so_path)
    except OSError:
        return
    if not hasattr(lib, "axon_start_nrt_profile"):
        return
    lib.axon_start_nrt_profile.argtypes = [
        ctypes.POINTER(ctypes.c_int64),
        ctypes.c_size_t,
    ]
    lib.axon_start_nrt_profile.restype = ctypes.c_int64
    lib.axon_stop_nrt_profile.argtypes = [ctypes.c_char_p]
    lib.axon_stop_nrt_profile.restype = ctypes.c_int64

    @contextlib.contextmanager
    def _hook(output_dir, device_ids):
        import jax

        jax.devices()
        if device_ids:
            ids = (ctypes.c_int64 * len(device_ids))(*device_ids)
            rc = lib.axon_start_nrt_profile(ids, len(device_ids))
        else:
            rc = lib.axon_start_nrt_profile(None, 0)
        if rc != 0:
            raise RuntimeError(f"axon_start_nrt_profile rc={rc}")
        try:
            yield
        finally:
            n = lib.axon_stop_nrt_profile(str(output_dir).encode())
            if n < 0:
                raise RuntimeError(f"axon_stop_nrt_profile rc={n}")

    set_axon_ntff_profile_hook(_hook)


def _patch_upload_artifacts():
    import concourse.bass_utils as bu

    if getattr(bu.upload_artifacts, "_safe", False):
        return
    orig = bu.upload_artifacts

    def safe(tmpdir):
        try:
            return orig(tmpdir)
        except Exception:
            return tmpdir

    safe._safe = True
    bu.upload_artifacts = safe


def _act_recip(scalar_eng, nc, out_ap, in_ap):
    """Directly emit Activation(Reciprocal) (the nc.scalar.activation wrapper
    rejects Reciprocal)."""
    ins = [scalar_eng.lower_ap(in_ap)]
    for argv in (0.0, 1.0, 0.0):  # bias, scale, alpha
        ins.append(mybir.ImmediateValue(dtype=F32, value=argv))
    return scalar_eng.add_instruction(
        mybir.InstActivation(
            name=nc.get_next_instruction_name(),
            func=mybir.ActivationFunctionType.Reciprocal,
            ins=ins,
            outs=[scalar_eng.lower_ap(out_ap)],
        )
    )


class _Ticks:
    """Per-engine completion tick bookkeeping for cross-engine waits.

    Tick numbers are precomputed from the schedule (so any engine stream can
    be emitted first); each tracked instruction gets `.then_inc(sem, 1)`;
    waiters use wait_ge(sem, tick).
    """

    def __init__(self, orders, sems):
        # orders: {eng_name: [key, ...]} in emission order; sems: {eng: sem}
        self.tick_no = {}
        self.key_eng = {}
        for eng, keys in orders.items():
            for t, key in enumerate(keys, start=1):
                self.tick_no[key] = t
                self.key_eng[key] = eng
        self.sems = sems

    def mark(self, inst, key):
        inst.then_inc(self.sems[self.key_eng[key]], 1)

    def wait(self, engine, key):
        engine.wait_ge(self.sems[self.key_eng[key]], self.tick_no[key])


def _build_graph():
    nc = bass.Bass()
    A_ext = nc.declare_dram_parameter("anchors_p", [P, BLOCKS * 4], F32, isOutput=False)
    AR_ext = nc.declare_dram_parameter("aarea", [P, BLOCKS], F32, isOutput=False)
    GT_ext = nc.declare_dram_parameter("gtrows", [5, NPAIR], F32, isOutput=False)
    out_ext = nc.declare_dram_parameter("out", [P, BLOCKS * 4], F32, isOutput=True)

    Alu = mybir.AluOpType
    NB = 2  # cross-engine buffer depth

    with (
        nc.sbuf_tensor("A", [P, BLOCKS * 4], F32) as A,
        nc.sbuf_tensor("AR", [P, BLOCKS], F32) as AR,
        nc.sbuf_tensor("GTB", [P, 5, NPAIR], F32) as GTB,
        nc.sbuf_tensor("TX", [P, NPAIR], F32) as TX,
        nc.sbuf_tensor("TY", [P, NPAIR], F32) as TY,
        nc.sbuf_tensor("SX", [P, NB, NPAIR], F32) as SXb,
        nc.sbuf_tensor("SY", [P, NB, NPAIR], F32) as SYb,
        nc.sbuf_tensor("IHR", [P, NB, NPAIR], F32) as IHRb,
        nc.sbuf_tensor("INT", [P, NB, NPAIR], F32) as INTb,
        nc.sbuf_tensor("RI", [P, NB, NPAIR], F32) as RIb,
        nc.sbuf_tensor("SA", [P, NB, NPAIR], F32) as SAb,
        nc.sbuf_tensor("VB", [P, NB, NPAIR], F32) as VBb,
        nc.sbuf_tensor("MH", [P, NB, NPAIR // 2], F32) as MHb,
        nc.sbuf_tensor("VOUT", [P, BLOCKS * 4], F32) as VOUT,
        nc.sbuf_tensor("MIOU", [P, BLOCKS * 4], F32) as MIOU,
        nc.Block() as block,
        nc.semaphore("dma_sem") as dma_sem,
        nc.semaphore("dve_sem") as dve_sem,
        nc.semaphore("act_sem") as act_sem,
        nc.semaphore("pool_sem") as pool_sem,
    ):
        # ---- schedule (must mirror the emission loops below exactly) ----
        dve_order = []
        for s in range(BLOCKS + 3):
            if s < BLOCKS:
                dve_order.append(("sy", s))
            if s >= 1 and s - 1 < BLOCKS:
                dve_order.append(("inter", s - 1))
            if s >= 3 and s - 3 < BLOCKS:
                dve_order.append(("red", s - 3))
        dve_order.append(("vc", 0))
        act_order = []
        for s in range(BLOCKS + 2):
            if s >= 1 and s - 1 < BLOCKS:
                act_order.append(("ihr", s - 1))
                act_order.append(("sa", s - 1))
            if s >= 2 and s - 2 < BLOCKS:
                act_order.append(("rint", s - 2))
        act_order.append(("miou", 0))
        pool_order = []
        for s in range(BLOCKS + 2):
            if s >= 2 and s - 2 < BLOCKS:
                pool_order.append(("v", s - 2))

        tk = _Ticks(
            {"dve": dve_order, "act": act_order, "pool": pool_order},
            {"dve": dve_sem, "act": act_sem, "pool": pool_sem},
        )

        @block.sync
        def _(sync):
            sync.dma_start(out=A[:, :], in_=A_ext[:, :]).then_inc(dma_sem, 16)
            sync.dma_start(out=AR[:, :], in_=AR_ext[:, :]).then_inc(dma_sem, 16)
            g_ap = GT_ext[:, :]
            g_b = bass.AP(
                tensor=g_ap.tensor, offset=g_ap.offset, ap=[[0, P]] + list(g_ap.ap)
            )
            sync.dma_start(out=GTB[:, :, :], in_=g_b).then_inc(dma_sem, 16)

        GX1N = GTB[:, 0, :]
        GX2 = GTB[:, 1, :]
        GY1N = GTB[:, 2, :]
        GY2 = GTB[:, 3, :]
        GAREA = GTB[:, 4, :]

        # ---- emission: three engine streams, software-pipelined ----
        # stage offsets at "step" s (s = 0..BLOCKS+3):
        #   DVE: front(s)               [tx,sx,ty,sy]
        #   ACT: ihr(s-1), SA(s-1)
        #   DVE: inter(s-1)
        #   ACT: rint(s-2)
        #   Pool: v(s-2), mh(s-2)
        #   DVE: red(s-3)
        # Emission is per-engine (whole stream at once); waits use tick map.
        dve_prog = []
        act_prog = []
        pool_prog = []

        def dve_front(vector, j):
            nax1 = A[:, 4 * j + 0 : 4 * j + 1]
            ax2 = A[:, 4 * j + 1 : 4 * j + 2]
            nay1 = A[:, 4 * j + 2 : 4 * j + 3]
            ay2 = A[:, 4 * j + 3 : 4 * j + 4]
            b = j % NB
            if j >= NB:
                tk.wait(vector, ("ihr", j - NB))  # ACT done reading SX/SY slot
            vector.tensor_scalar(
                out=TX[:, :], in0=GX2, scalar1=ax2, scalar2=None, op0=Alu.min
            )
            vector.scalar_tensor_tensor(
                out=SXb[:, b, :], in0=GX1N, scalar=nax1, in1=TX[:, :],
                op0=Alu.min, op1=Alu.add,
            )
            vector.tensor_scalar(
                out=TY[:, :], in0=GY2, scalar1=ay2, scalar2=None, op0=Alu.min
            )
            i = vector.scalar_tensor_tensor(
                out=SYb[:, b, :], in0=GY1N, scalar=nay1, in1=TY[:, :],
                op0=Alu.min, op1=Alu.add,
            )
            tk.mark(i, ("sy", j))

        def dve_inter(vector, j):
            b = j % NB
            tk.wait(vector, ("ihr", j))
            if j >= NB:
                tk.wait(vector, ("rint", j - NB))  # ACT done reading INT slot
            i = vector.scalar_tensor_tensor(
                out=INTb[:, b, :], in0=SXb[:, b, :], scalar=0.0,
                in1=IHRb[:, b, :], op0=Alu.max, op1=Alu.mult,
            )
            tk.mark(i, ("inter", j))

        def dve_red(vector, j):
            b = j % NB
            tk.wait(vector, ("v", j))
            i = vector.tensor_reduce(
                out=VOUT[:, 4 * j : 4 * (j + 1)],
                in_=VBb[:, b, :].rearrange("p (bt g) -> p bt g", bt=BATCH),
                axis=mybir.AxisListType.X,
                op=Alu.min,
            )
            tk.mark(i, ("red", j))

        def act_mid(scalar, j):
            b = j % NB
            tk.wait(scalar, ("sy", j))
            if j >= NB:
                tk.wait(scalar, ("inter", j - NB))  # DVE done reading IHR slot
            i = scalar.activation(
                out=IHRb[:, b, :], in_=SYb[:, b, :],
                func=mybir.ActivationFunctionType.Relu,
            )
            tk.mark(i, ("ihr", j))
            if j >= NB:
                tk.wait(scalar, ("v", j - NB))  # Pool done reading SA slot
            i = scalar.activation(
                out=SAb[:, b, :], in_=GAREA,
                func=mybir.ActivationFunctionType.Identity,
                bias=AR[:, j : j + 1], scale=1.0,
            )
            tk.mark(i, ("sa", j))

        def act_rint(scalar, j):
            b = j % NB
            tk.wait(scalar, ("inter", j))
            if j >= NB:
                tk.wait(scalar, ("v", j - NB))  # Pool done reading RI slot
            i = _act_recip(scalar, nc, RIb[:, b, :], INTb[:, b, :])
            tk.mark(i, ("rint", j))

        def pool_v(gpsimd, j):
            b = j % NB
            tk.wait(gpsimd, ("rint", j))
            tk.wait(gpsimd, ("sa", j))
            if j >= NB:
                tk.wait(gpsimd, ("red", j - NB))  # DVE done reading VB slot
            i = gpsimd.tensor_tensor(
                out=VBb[:, b, :], in0=SAb[:, b, :], in1=RIb[:, b, :], op=Alu.mult
            )
            tk.mark(i, ("v", j))

        @block.vector
        def _(vector):
            vector.wait_ge(dma_sem, 48)
            for s in range(BLOCKS + 3):
                if s < BLOCKS:
                    dve_front(vector, s)
                if 1 <= s + 0 and s - 1 < BLOCKS and s >= 1:
                    dve_inter(vector, s - 1)
                if s >= 3 and s - 3 < BLOCKS:
                    dve_red(vector, s - 3)
            i = vector.tensor_scalar(
                out=MIOU[:, :], in0=VOUT[:, :], scalar1=1e30, scalar2=-1.0,
                op0=Alu.min, op1=Alu.add,
            )
            tk.mark(i, ("vc", 0))

        @block.scalar
        def _(scalar):
            for s in range(BLOCKS + 2):
                if s >= 1 and s - 1 < BLOCKS:
                    act_mid(scalar, s - 1)
                if s >= 2 and s - 2 < BLOCKS:
                    act_rint(scalar, s - 2)
            tk.wait(scalar, ("vc", 0))
            i = _act_recip(scalar, nc, MIOU[:, :], MIOU[:, :])
            tk.mark(i, ("miou", 0))

        @block.gpsimd
        def _(gpsimd):
            for s in range(BLOCKS + 2):
                if s >= 2 and s - 2 < BLOCKS:
                    pool_v(gpsimd, s - 2)

        @block.sync
        def _(sync):
            tk.wait(sync, ("miou", 0))
            sync.dma_start(out=out_ext[:, :], in_=MIOU[:, :]).then_inc(dma_sem, 16)
            sync.wait_ge(dma_sem, 64)

    return nc


def kernel(anchors: np.ndarray, gt_boxes: np.ndarray) -> np.ndarray:
    global LAST_EXEC_NS
    anchors = np.asarray(anchors, dtype=np.float32)
    gt_boxes = np.asarray(gt_boxes, dtype=np.float32)

    apad = np.zeros((N_PAD, 4), dtype=np.float32)
    apad[:N_ANCHORS] = anchors

    g = gt_boxes.reshape(NPAIR, 4)
    garea = (g[:, 2] - g[:, 0]) * (g[:, 3] - g[:, 1])
    gtrows = np.stack([-g[:, 0], g[:, 2], -g[:, 1], g[:, 3], garea]).astype(np.float32)
    gtrows = np.ascontiguousarray(gtrows)

    in_maps = []
    for c in range(N_CORES):
        sh = apad[c * N_LOC : (c + 1) * N_LOC]
        a3 = sh.reshape(P, BLOCKS, 4)
        ap = np.empty_like(a3)
        ap[:, :, 0] = -a3[:, :, 0]
        ap[:, :, 1] = a3[:, :, 2]
        ap[:, :, 2] = -a3[:, :, 1]
        ap[:, :, 3] = a3[:, :, 3]
        aarea = (a3[:, :, 2] - a3[:, :, 0]) * (a3[:, :, 3] - a3[:, :, 1])
        in_maps.append(
            {
                "anchors_p": np.ascontiguousarray(ap.reshape(P, BLOCKS * 4)),
                "aarea": np.ascontiguousarray(aarea.astype(np.float32)),
                "gtrows": gtrows,
            }
        )

    nc = _build_graph()
    trace = os.environ.get("ANCHOR_TRACE", "0") == "1"
    core_ids = list(range(N_CORES))
    if trace:
        _ensure_axon_ntff_hook()
        _patch_upload_artifacts()
        try:
            res = run_bass_kernel_spmd(nc, in_maps, core_ids=core_ids, trace=True)
        except Exception as e:
            print(f"trace run failed ({type(e).__name__}: {e}); falling back", file=sys.stderr)
            res = run_bass_kernel_spmd(nc, in_maps, core_ids=core_ids, trace=False)
    else:
        res = run_bass_kernel_spmd(nc, in_maps, core_ids=core_ids, trace=False)
    LAST_EXEC_NS = res.exec_time_ns

    out = np.empty((BATCH, N_PAD), dtype=np.float32)
    for c in range(N_CORES):
        o = res.results[c]["out"].reshape(P, BLOCKS, 4)
        out[:, c * N_LOC : (c + 1) * N_LOC] = o.transpose(2, 0, 1).reshape(BATCH, N_LOC)
    return out[:, :N_ANCHORS]


# revision 27
# speedup vs baseline: 1.3306x; 1.1890x over previous
"""AnchorTargetLayer max-IoU kernel for 8 TRN2 NeuronCores.

max_iou[b, n] = max_g IoU(anchor_n, gt_box[b, g]);
anchors [100000, 4] f32, gt_boxes [4, 64, 4] f32 -> out [4, 100000] f32.

Sharding: anchors split 8 ways (12544/core incl pad), gt replicated,
no collectives. Per-core layout: anchors on SBUF partitions (128 per
block, 98 blocks), all B*G = 256 (batch, gt) pairs on the free dim.
GT-derived rows are broadcast once into [128, 5*256] SBUF (loop
invariant); per-anchor coords are [128,1] per-partition scalars.

Math per (anchor, pair):
  iw   = min(ax2, gx2) + min(-ax1, -gx1)        (x-overlap, may be <0)
  ih   = likewise in y
  inter = relu(iw) * relu(ih)
  v    = (areaA + areaG) * (1/inter)            (= union/inter + 1)
  vmin[b] = min_g v ;  out = 1/(min(vmin,1e30) - 1)
(inter == 0 -> Reciprocal gives inf -> dropped by the min; an anchor
with no overlap at all ends at exactly 0 via the 1e30 clamp.)

Engine split, software-pipelined with per-instruction semaphore ticks:
  Vector (DVE):  tx, sx, ty, sy (the two 1-D overlap chains),
                 inter = relu(sx)*ihr, and the per-batch min reduce
  Scalar (ACT):  ihr = Relu(sy), SA = areaA+areaG (Identity+bias),
                 rint = Reciprocal(inter)  [all share one table set]
  GpSimd (Pool): v = SA * rint  (tensor_tensor mult)
DVE is the bottleneck (~90% busy); ACT ~60%, Pool ~35%.
"""

import os
import sys

import numpy as np

sys.path.insert(0, "/opt/trn_rl_repo")

import concourse.bass as bass
import concourse.mybir as mybir
from concourse.bass_utils import run_bass_kernel_spmd

N_ANCHORS = 100000
BATCH = 4
N_GT = 64
N_CORES = 8

P = 128
BLOCKS = 98
N_LOC = P * BLOCKS          # 12544
N_PAD = N_LOC * N_CORES     # 100352
NPAIR = BATCH * N_GT        # 256

F32 = mybir.dt.float32
EPS = 1e-15

LAST_EXEC_NS = None


def _ensure_axon_ntff_hook():
    try:
        import antenv.axon_hooks  # noqa: F401

        return
    except ImportError:
        pass
    import contextlib
    import ctypes
    import types

    import antenv

    m = types.ModuleType("antenv.axon_hooks")
    m._hook = None

    def set_axon_ntff_profile_hook(h):
        m._hook = h

    def get_axon_ntff_profile_hook():
        return m._hook

    m.set_axon_ntff_profile_hook = set_axon_ntff_profile_hook
    m.get_axon_ntff_profile_hook = get_axon_ntff_profile_hook
    sys.modules["antenv.axon_hooks"] = m
    antenv.axon_hooks = m

    so_path = os.environ.get("PJRT_LIBRARY_PATH", "/opt/axon/libaxon_pjrt.so")
    try:
        lib = ctypes.CDLL(so_path)
    except OSError:
        return
    if not hasattr(lib, "axon_start_nrt_profile"):
        return
    lib.axon_start_nrt_profile.argtypes = [
        ctypes.POINTER(ctypes.c_int64),
        ctypes.c_size_t,
    ]
    lib.axon_start_nrt_profile.restype = ctypes.c_int64
    lib.axon_stop_nrt_profile.argtypes = [ctypes.c_char_p]
    lib.axon_stop_nrt_profile.restype = ctypes.c_int64

    @contextlib.contextmanager
    def _hook(output_dir, device_ids):
        import jax

        jax.devices()
        if device_ids:
            ids = (ctypes.c_int64 * len(device_ids))(*device_ids)
            rc = lib.axon_start_nrt_profile(ids, len(device_ids))
        else:
            rc = lib.axon_start_nrt_profile(None, 0)
        if rc != 0:
            raise RuntimeError(f"axon_start_nrt_profile rc={rc}")
        try:
            yield
        finally:
            n = lib.axon_stop_nrt_profile(str(output_dir).encode())
            if n < 0:
                raise RuntimeError(f"axon_stop_nrt_profile rc={n}")

    set_axon_ntff_profile_hook(_hook)


def _patch_upload_artifacts():
    import concourse.bass_utils as bu

    if getattr(bu.upload_artifacts, "_safe", False):
        return
    orig = bu.upload_artifacts

    def safe(tmpdir):
        try:
            return orig(tmpdir)
        except Exception:
            return tmpdir

    safe._safe = True
    bu.upload_artifacts = safe


def _act_recip(scalar_eng, nc, out_ap, in_ap):
    """Directly emit Activation(Reciprocal) (the nc.scalar.activation wrapper
    rejects Reciprocal)."""
    ins = [scalar_eng.lower_ap(in_ap)]
    for argv in (0.0, 1.0, 0.0):  # bias, scale, alpha
        ins.append(mybir.ImmediateValue(dtype=F32, value=argv))
    return scalar_eng.add_instruction(
        mybir.InstActivation(
            name=nc.get_next_instruction_name(),
            func=mybir.ActivationFunctionType.Reciprocal,
            ins=ins,
            outs=[scalar_eng.lower_ap(out_ap)],
        )
    )


class _Ticks:
    """Per-engine completion tick bookkeeping for cross-engine waits.

    Tick numbers are precomputed from the schedule (so any engine stream can
    be emitted first); each tracked instruction gets `.then_inc(sem, 1)`;
    waiters use wait_ge(sem, tick).
    """

    def __init__(self, orders, sems):
        # orders: {eng_name: [key, ...]} in emission order; sems: {eng: sem}
        self.tick_no = {}
        self.key_eng = {}
        for eng, keys in orders.items():
            for t, key in enumerate(keys, start=1):
                self.tick_no[key] = t
                self.key_eng[key] = eng
        self.sems = sems

    def mark(self, inst, key):
        inst.then_inc(self.sems[self.key_eng[key]], 1)

    def wait(self, engine, key):
        engine.wait_ge(self.sems[self.key_eng[key]], self.tick_no[key])


def _build_graph():
    nc = bass.Bass()
    A_ext = nc.declare_dram_parameter("anchors_p", [P, BLOCKS * 4], F32, isOutput=False)
    AR_ext = nc.declare_dram_parameter("aarea", [P, BLOCKS], F32, isOutput=False)
    GT_ext = nc.declare_dram_parameter("gtrows", [5, NPAIR], F32, isOutput=False)
    out_ext = nc.declare_dram_parameter("out", [P, BLOCKS * 4], F32, isOutput=True)

    Alu = mybir.AluOpType
    NB = 2  # cross-engine buffer depth

    with (
        nc.sbuf_tensor("A", [P, BLOCKS * 4], F32) as A,
        nc.sbuf_tensor("AR", [P, BLOCKS], F32) as AR,
        nc.sbuf_tensor("GTB", [P, 5, NPAIR], F32) as GTB,
        nc.sbuf_tensor("TX", [P, NPAIR], F32) as TX,
        nc.sbuf_tensor("TY", [P, NPAIR], F32) as TY,
        nc.sbuf_tensor("SX", [P, NB, NPAIR], F32) as SXb,
        nc.sbuf_tensor("SY", [P, NB, NPAIR], F32) as SYb,
        nc.sbuf_tensor("IHR", [P, NB, NPAIR], F32) as IHRb,
        nc.sbuf_tensor("INT", [P, NB, NPAIR], F32) as INTb,
        nc.sbuf_tensor("RI", [P, NB, NPAIR], F32) as RIb,
        nc.sbuf_tensor("SA", [P, NB, NPAIR], F32) as SAb,
        nc.sbuf_tensor("VB", [P, NB, NPAIR], F32) as VBb,
        nc.sbuf_tensor("MH", [P, NB, NPAIR // 2], F32) as MHb,
        nc.sbuf_tensor("VOUT", [P, BLOCKS * 4], F32) as VOUT,
        nc.sbuf_tensor("MIOU", [P, BLOCKS * 4], F32) as MIOU,
        nc.Block() as block,
        nc.semaphore("dma_sem") as dma_sem,
        nc.semaphore("dve_sem") as dve_sem,
        nc.semaphore("act_sem") as act_sem,
        nc.semaphore("pool_sem") as pool_sem,
    ):
        # ---- schedule (must mirror the emission loops below exactly) ----
        dve_order = []
        for s in range(BLOCKS + 3):
            if s < BLOCKS:
                dve_order.append(("sy", s))
            if s >= 1 and s - 1 < BLOCKS:
                dve_order.append(("inter", s - 1))
            if s >= 3 and s - 3 < BLOCKS:
                dve_order.append(("red", s - 3))
        dve_order.append(("vc", 0))
        act_order = []
        for s in range(BLOCKS + 2):
            if s >= 1 and s - 1 < BLOCKS:
                act_order.append(("ihr", s - 1))
                act_order.append(("sa", s - 1))
            if s >= 2 and s - 2 < BLOCKS:
                act_order.append(("rint", s - 2))
        act_order.append(("miou", 0))
        pool_order = []
        for s in range(BLOCKS + 2):
            if s >= 2 and s - 2 < BLOCKS:
                pool_order.append(("v", s - 2))

        tk = _Ticks(
            {"dve": dve_order, "act": act_order, "pool": pool_order},
            {"dve": dve_sem, "act": act_sem, "pool": pool_sem},
        )

        @block.sync
        def _(sync):
            sync.dma_start(out=A[:, :], in_=A_ext[:, :]).then_inc(dma_sem, 16)
            sync.dma_start(out=AR[:, :], in_=AR_ext[:, :]).then_inc(dma_sem, 16)
            g_ap = GT_ext[:, :]
            g_b = bass.AP(
                tensor=g_ap.tensor, offset=g_ap.offset, ap=[[0, P]] + list(g_ap.ap)
            )
            sync.dma_start(out=GTB[:, :, :], in_=g_b).then_inc(dma_sem, 16)

        GX1N = GTB[:, 0, :]
        GX2 = GTB[:, 1, :]
        GY1N = GTB[:, 2, :]
        GY2 = GTB[:, 3, :]
        GAREA = GTB[:, 4, :]

        # ---- emission: three engine streams, software-pipelined ----
        # stage offsets at "step" s (s = 0..BLOCKS+3):
        #   DVE: front(s)               [tx,sx,ty,sy]
        #   ACT: ihr(s-1), SA(s-1)
        #   DVE: inter(s-1)
        #   ACT: rint(s-2)
        #   Pool: v(s-2), mh(s-2)
        #   DVE: red(s-3)
        # Emission is per-engine (whole stream at once); waits use tick map.
        dve_prog = []
        act_prog = []
        pool_prog = []

        def dve_front(vector, j):
            nax1 = A[:, 4 * j + 0 : 4 * j + 1]
            ax2 = A[:, 4 * j + 1 : 4 * j + 2]
            nay1 = A[:, 4 * j + 2 : 4 * j + 3]
            ay2 = A[:, 4 * j + 3 : 4 * j + 4]
            b = j % NB
            if j >= NB:
                tk.wait(vector, ("ihr", j - NB))  # ACT done reading SX/SY slot
            vector.tensor_scalar(
                out=TX[:, :], in0=GX2, scalar1=ax2, scalar2=None, op0=Alu.min
            )
            vector.scalar_tensor_tensor(
                out=SXb[:, b, :], in0=GX1N, scalar=nax1, in1=TX[:, :],
                op0=Alu.min, op1=Alu.add,
            )
            vector.tensor_scalar(
                out=TY[:, :], in0=GY2, scalar1=ay2, scalar2=None, op0=Alu.min
            )
            i = vector.scalar_tensor_tensor(
                out=SYb[:, b, :], in0=GY1N, scalar=nay1, in1=TY[:, :],
                op0=Alu.min, op1=Alu.add,
            )
            tk.mark(i, ("sy", j))

        def dve_inter(vector, j):
            b = j % NB
            tk.wait(vector, ("ihr", j))
            if j >= NB:
                tk.wait(vector, ("rint", j - NB))  # ACT done reading INT slot
            i = vector.scalar_tensor_tensor(
                out=INTb[:, b, :], in0=SXb[:, b, :], scalar=0.0,
                in1=IHRb[:, b, :], op0=Alu.max, op1=Alu.mult,
            )
            tk.mark(i, ("inter", j))

        def dve_red(vector, j):
            b = j % NB
            tk.wait(vector, ("v", j))
            i = vector.tensor_reduce(
                out=VOUT[:, 4 * j : 4 * (j + 1)],
                in_=VBb[:, b, :].rearrange("p (bt g) -> p bt g", bt=BATCH),
                axis=mybir.AxisListType.X,
                op=Alu.min,
            )
            tk.mark(i, ("red", j))

        def act_mid(scalar, j):
            b = j % NB
            tk.wait(scalar, ("sy", j))
            if j >= NB:
                tk.wait(scalar, ("inter", j - NB))  # DVE done reading IHR slot
            i = scalar.activation(
                out=IHRb[:, b, :], in_=SYb[:, b, :],
                func=mybir.ActivationFunctionType.Relu,
            )
            tk.mark(i, ("ihr", j))
            if j >= NB:
                tk.wait(scalar, ("v", j - NB))  # Pool done reading SA slot
            i = scalar.activation(
                out=SAb[:, b, :], in_=GAREA,
                func=mybir.ActivationFunctionType.Identity,
                bias=AR[:, j : j + 1], scale=1.0,
            )
            tk.mark(i, ("sa", j))

        def act_rint(scalar, j):
            b = j % NB
            tk.wait(scalar, ("inter", j))
            if j >= NB:
                tk.wait(scalar, ("v", j - NB))  # Pool done reading RI slot
            i = _act_recip(scalar, nc, RIb[:, b, :], INTb[:, b, :])
            tk.mark(i, ("rint", j))

        def pool_v(gpsimd, j):
            b = j % NB
            tk.wait(gpsimd, ("rint", j))
            tk.wait(gpsimd, ("sa", j))
            if j >= NB:
                tk.wait(gpsimd, ("red", j - NB))  # DVE done reading VB slot
            i = gpsimd.tensor_tensor(
                out=VBb[:, b, :], in0=SAb[:, b, :], in1=RIb[:, b, :], op=Alu.mult
            )
            tk.mark(i, ("v", j))

        @block.vector
        def _(vector):
            vector.wait_ge(dma_sem, 48)
            for s in range(BLOCKS + 3):
                if s < BLOCKS:
                    dve_front(vector, s)
                if 1 <= s + 0 and s - 1 < BLOCKS and s >= 1:
                    dve_inter(vector, s - 1)
                if s >= 3 and s - 3 < BLOCKS:
                    dve_red(vector, s - 3)
            i = vector.tensor_scalar(
                out=MIOU[:, :], in0=VOUT[:, :], scalar1=1e30, scalar2=-1.0,
                op0=Alu.min, op1=Alu.add,
            )
            tk.mark(i, ("vc", 0))

        @block.scalar
        def _(scalar):
            for s in range(BLOCKS + 2):
                if s >= 1 and s - 1 < BLOCKS:
                    act_mid(scalar, s - 1)
                if s >= 2 and s - 2 < BLOCKS:
                    act_rint(scalar, s - 2)
            tk.wait(scalar, ("vc", 0))
            i = _act_recip(scalar, nc, MIOU[:, :], MIOU[:, :])
            tk.mark(i, ("miou", 0))

        @block.gpsimd
        def _(gpsimd):
            for s in range(BLOCKS + 2):
                if s >= 2 and s - 2 < BLOCKS:
                    pool_v(gpsimd, s - 2)

        @block.sync
        def _(sync):
            tk.wait(sync, ("miou", 0))
            sync.dma_start(out=out_ext[:, :], in_=MIOU[:, :]).then_inc(dma_sem, 16)
            sync.wait_ge(dma_sem, 64)

    return nc


def kernel(anchors: np.ndarray, gt_boxes: np.ndarray) -> np.ndarray:
    global LAST_EXEC_NS
    anchors = np.asarray(anchors, dtype=np.float32)
    gt_boxes = np.asarray(gt_boxes, dtype=np.float32)

    apad = np.zeros((N_PAD, 4), dtype=np.float32)
    apad[:N_ANCHORS] = anchors

    g = gt_boxes.reshape(NPAIR, 4)
    garea = (g[:, 2] - g[:, 0]) * (g[:, 3] - g[:, 1])
    gtrows = np.stack([-g[:, 0], g[:, 2], -g[:, 1], g[:, 3], garea]).astype(np.float32)
    gtrows = np.ascontiguousarray(gtrows)

    in_maps = []
    for c in range(N_CORES):
        sh = apad[c * N_LOC : (c + 1) * N_LOC]
        a3 = sh.reshape(P, BLOCKS, 4)
        ap = np.empty_like(a3)
        ap[:, :, 0] = -a3[:, :, 0]
        ap[:, :, 1] = a3[:, :, 2]
        ap[:, :, 2] = -a3[:, :, 1]
        ap[:, :, 3] = a3[:, :, 3]
        aarea = (a3[:, :, 2] - a3[:, :, 0]) * (a3[:, :, 3] - a3[:, :, 1])
        in_maps.append(
            {
                "anchors_p": np.ascontiguousarray(ap.reshape(P, BLOCKS * 4)),
                "aarea": np.ascontiguousarray(aarea.astype(np.float32)),
                "gtrows": gtrows,
            }
        )

    nc = _build_graph()
    trace = os.environ.get("ANCHOR_TRACE", "0") == "1"
    core_ids = list(range(N_CORES))
    if trace:
        _ensure_axon_ntff_hook()
        _patch_upload_artifacts()
        try:
            res = run_bass_kernel_spmd(nc, in_maps, core_ids=core_ids, trace=True)
        except Exception as e:
            print(f"trace run failed ({type(e).__name__}: {e}); falling back", file=sys.stderr)
            res = run_bass_kernel_spmd(nc, in_maps, core_ids=core_ids, trace=False)
    else:
        res = run_bass_kernel_spmd(nc, in_maps, core_ids=core_ids, trace=False)
    LAST_EXEC_NS = res.exec_time_ns

    out = np.empty((BATCH, N_PAD), dtype=np.float32)
    for c in range(N_CORES):
        o = res.results[c]["out"].reshape(P, BLOCKS, 4)
        out[:, c * N_LOC : (c + 1) * N_LOC] = o.transpose(2, 0, 1).reshape(BATCH, N_LOC)
    return out[:, :N_ANCHORS]
